# revision 1
# baseline (speedup 1.0000x reference)
"""Trainium2 Bass kernel for CustomEmbedding (embedding lookup with 16
override rows at the top of the vocab).

Semantics (matches the reference):
    out[b, s] = extra[input_ids[b, s] - 127984]  if input_ids[b, s] >= 127984
                weight[input_ids[b, s]]          otherwise

Sharding: data parallel over the batch dim — core c handles input_ids[c]
(4096 tokens); weight replicated.

Device kernel uses the production SWDGE gather/scatter ucode path
(dma_gather / dma_scatter_add, int16 indices), which requires indices
< 32768. The host splits the 128000-row table into 4 banks of 32768 rows,
sorts each core's tokens by bank (stable), and packs per-slot index lists:
  - gather slot s: 128 bank-local row indices (padded with row 0)
  - scatter slot s: the tokens' original positions (padded with a dummy
    row S, an extra scratch row of the output)
Per slot the device gathers 128 rows into SBUF and scatter-adds them to
their original output rows (output is zero-initialized by the bass2jax
donation path, so add == write).  Reserved ids (>= 127984) and any tokens
that exceed the static slot budget are fixed up on the host afterwards.
"""

import sys

if "/opt/trn_rl_repo" not in sys.path:
    sys.path.insert(0, "/opt/trn_rl_repo")

import numpy as np

import concourse.tile as tile
from concourse import bacc, mybir
from concourse.bass_utils import run_bass_kernel_spmd

VOCAB = 128000
DIM = 2048
B, S = 8, 4096
N_CORES = 8
N_OVER = 16
OVER_START = VOCAB - N_OVER  # 127984
P = 128

BANK_ROWS = 1 << 15  # 32768 — int16-addressable rows per gather bank
N_BANKS = 4
# Static per-bank slot budget (chunks of 128 tokens). Sized for the uniform
# reference distribution (~1049/1049/1049/950 tokens per bank per core →
# 9/9/9/8 chunks); overflow tokens fall back to the host fixup path.
SLOTS_PER_BANK = (9, 9, 9, 8)
N_SLOTS = sum(SLOTS_PER_BANK)
IDX_COLS = P // 16  # 8 free-dim columns per slot in the 16-partition wrap

DATA_BUFS = 4

_NC_CACHE = {}


def _build_nc(data_bufs=DATA_BUFS, reps=1):
    key = (data_bufs, reps)
    if key in _NC_CACHE:
        return _NC_CACHE[key]

    nc = bacc.Bacc(
        "TRN2", target_bir_lowering=False, debug=False, num_swdge_queues=4
    )
    weight = nc.dram_tensor(
        "weight", [VOCAB, DIM], mybir.dt.float32, kind="ExternalInput"
    )
    gidx = nc.dram_tensor(
        "gidx", [P, N_SLOTS * IDX_COLS], mybir.dt.int16, kind="ExternalInput"
    )
    sidx = nc.dram_tensor(
        "sidx", [P, N_SLOTS * IDX_COLS], mybir.dt.int16, kind="ExternalInput"
    )
    # row S is a scratch row collecting the padding-token writes
    out = nc.dram_tensor("out", [S + 1, DIM], mybir.dt.float32, kind="ExternalOutput")

    bank_aps = []
    for b in range(N_BANKS):
        hi = min((b + 1) * BANK_ROWS, VOCAB)
        bank_aps.append(weight.ap()[b * BANK_ROWS : hi])

    with tile.TileContext(nc) as tc:
        with (
            tc.tile_pool(name="idx", bufs=1) as idx_pool,
            tc.tile_pool(name="data", bufs=data_bufs) as data_pool,
        ):
            gsb = idx_pool.tile([P, N_SLOTS * IDX_COLS], mybir.dt.int16)
            nc.sync.dma_start(out=gsb[:], in_=gidx.ap())
            ssb = idx_pool.tile([P, N_SLOTS * IDX_COLS], mybir.dt.int16)
            nc.sync.dma_start(out=ssb[:], in_=sidx.ap())

            for _ in range(reps):
                s = 0
                for b in range(N_BANKS):
                    for _j in range(SLOTS_PER_BANK[b]):
                        t = data_pool.tile([P, 1, DIM], mybir.dt.float32)
                        nc.gpsimd.dma_gather(
                            t[:],
                            bank_aps[b],
                            gsb[:, s * IDX_COLS : (s + 1) * IDX_COLS],
                            P,
                            P,
                            DIM,
                            queue_num=0,
                        )
                        nc.gpsimd.dma_scatter_add(
                            out.ap(),
                            t[:],
                            ssb[:, s * IDX_COLS : (s + 1) * IDX_COLS],
                            P,
                            P,
                            DIM,
                            queue_num=0,
                        )
                        s += 1

    nc.compile()
    _NC_CACHE[key] = nc
    return nc


def _wrap16(a):
    """[N_SLOTS, 128] int16 -> [128, N_SLOTS*8]: idx i of slot s lands at
    (partition i%16, col s*8 + i//16), replicated to all 128 partitions."""
    blocks = a.reshape(N_SLOTS, IDX_COLS, 16).transpose(0, 2, 1)  # [S, 16, 8]
    flat = blocks.transpose(1, 0, 2).reshape(16, N_SLOTS * IDX_COLS)
    return np.ascontiguousarray(np.tile(flat, (8, 1)))


def _prep_core(ids_c):
    """Build gather/scatter index planes + host fixup list for one core."""
    bank = (ids_c >> 15).astype(np.int64)
    order = np.argsort(bank, kind="stable")
    gl = np.zeros((N_SLOTS, P), np.int16)      # pad: bank row 0 (valid)
    sl = np.full((N_SLOTS, P), S, np.int16)    # pad: dummy out row S
    fix = [np.where(ids_c >= OVER_START)[0]]   # reserved ids -> host fixup
    s0 = 0
    for b in range(N_BANKS):
        pos = order[bank[order] == b]
        spb = SLOTS_PER_BANK[b]
        if len(pos) > spb * P:  # static budget exceeded -> host fixup
            fix.append(pos[spb * P :])
            pos = pos[: spb * P]
        nch = (len(pos) + P - 1) // P
        for j in range(nch):
            pp = pos[j * P : (j + 1) * P]
            gl[s0 + j, : len(pp)] = (ids_c[pp] - (b << 15)).astype(np.int16)
            sl[s0 + j, : len(pp)] = pp.astype(np.int16)
        s0 += spb
    return _wrap16(gl), _wrap16(sl), np.unique(np.concatenate(fix))


def kernel(input_ids, weight, extra):
    input_ids = np.ascontiguousarray(np.asarray(input_ids), dtype=np.int32)
    weight = np.ascontiguousarray(np.asarray(weight), dtype=np.float32)
    extra = np.ascontiguousarray(np.asarray(extra), dtype=np.float32)
    assert input_ids.shape == (B, S), input_ids.shape
    assert weight.shape == (VOCAB, DIM), weight.shape
    assert extra.shape == (N_OVER, DIM), extra.shape

    nc = _build_nc()
    in_maps = []
    fixes = []
    for c in range(N_CORES):
        g, sdx, fix = _prep_core(input_ids[c])
        in_maps.append({"weight": weight, "gidx": g, "sidx": sdx})
        fixes.append(fix)

    res = run_bass_kernel_spmd(nc, in_maps, core_ids=list(range(N_CORES)))

    out = np.stack(
        [res.results[c]["out"][:S] for c in range(N_CORES)], axis=0
    )
    # host fixup: reserved ids + any slot-budget overflow
    for c in range(N_CORES):
        fix = fixes[c]
        if len(fix) == 0:
            continue
        ids_f = input_ids[c][fix]
        rows = np.where(
            (ids_f >= OVER_START)[:, None],
            extra[np.clip(ids_f - OVER_START, 0, N_OVER - 1)],
            weight[ids_f],
        )
        out[c][fix] = rows
    return out



# revision 2
# speedup vs baseline: 2.4751x; 2.4751x over previous
"""Trainium2 Bass kernel for CustomEmbedding (embedding lookup with 16
override rows at the top of the vocab).

Semantics (matches the reference):
    out[b, s] = extra[input_ids[b, s] - 127984]  if input_ids[b, s] >= 127984
                weight[input_ids[b, s]]          otherwise

Sharding: data parallel over the batch dim — core c handles input_ids[c]
(4096 tokens); the table is replicated per core (host-converted to f16:
the 2e-2 rel-err budget dwarfs f16's ~5e-4 quantization error, and f16
halves both HBM traffic and DMA descriptor payload).

Device kernel: production SWDGE gather/scatter ucode (dma_gather /
dma_scatter_add, int16 indices → table split into 4 banks of 32768 rows;
each core's tokens stable-sorted by bank). Work is issued in waves
round-robin across all 4 SWDGE queues, and each queue scatters into its
OWN DRAM output tensor so no two scatters touch the same tensor (avoids
the tile-framework WAW serialization chain). Rows are disjoint across the
4 outputs (each token lands in exactly one), so the host merge is a sum.
Reserved ids (>= 127984) and slot-budget overflow are fixed up on the
host in full f32 precision.
"""

import sys

if "/opt/trn_rl_repo" not in sys.path:
    sys.path.insert(0, "/opt/trn_rl_repo")

import numpy as np

import concourse.tile as tile
from concourse import bacc, mybir
from concourse.bass_utils import run_bass_kernel_spmd

VOCAB = 128000
DIM = 2048
B, S = 8, 4096
N_CORES = 8
N_OVER = 16
OVER_START = VOCAB - N_OVER  # 127984
P = 128

BANK_ROWS = 1 << 15  # 32768 — int16-addressable rows per gather bank
N_BANKS = 4
# Static per-bank slot budget (chunks of 128 tokens). Sized for the uniform
# reference distribution (~1049/1049/1049/950 tokens per bank per core →
# 9/9/9/8 chunks); overflow tokens fall back to the host fixup path.
SLOTS_PER_BANK = (9, 9, 9, 8)
N_SLOTS = sum(SLOTS_PER_BANK)
IDX_COLS = P // 16  # 8 free-dim columns per slot in the 16-partition wrap

N_QUEUES = 4
DATA_BUFS = 8

_NC_CACHE = {}


def _build_nc(data_bufs=DATA_BUFS, reps=1):
    key = (data_bufs, reps)
    if key in _NC_CACHE:
        return _NC_CACHE[key]

    nc = bacc.Bacc(
        "TRN2", target_bir_lowering=False, debug=False,
        num_swdge_queues=N_QUEUES,
    )
    wtab = nc.dram_tensor(
        "wtab", [VOCAB, DIM], mybir.dt.float16, kind="ExternalInput"
    )
    gidx = nc.dram_tensor(
        "gidx", [P, N_SLOTS * IDX_COLS], mybir.dt.int16, kind="ExternalInput"
    )
    sidx = nc.dram_tensor(
        "sidx", [P, N_SLOTS * IDX_COLS], mybir.dt.int16, kind="ExternalInput"
    )
    # one output per SWDGE queue; row S is a scratch row collecting the
    # padding-token writes. Host merges by summation (disjoint rows).
    outs = [
        nc.dram_tensor(
            f"out{q}", [S + 1, DIM], mybir.dt.float16, kind="ExternalOutput"
        )
        for q in range(N_QUEUES)
    ]

    bank_aps = []
    for b in range(N_BANKS):
        hi = min((b + 1) * BANK_ROWS, VOCAB)
        bank_aps.append(wtab.ap()[b * BANK_ROWS : hi])

    slot_bank = []
    for b in range(N_BANKS):
        slot_bank += [b] * SLOTS_PER_BANK[b]

    with tile.TileContext(nc) as tc:
        with (
            tc.tile_pool(name="idx", bufs=1) as idx_pool,
            tc.tile_pool(name="data", bufs=data_bufs) as data_pool,
        ):
            gsb = idx_pool.tile([P, N_SLOTS * IDX_COLS], mybir.dt.int16)
            nc.sync.dma_start(out=gsb[:], in_=gidx.ap())
            ssb = idx_pool.tile([P, N_SLOTS * IDX_COLS], mybir.dt.int16)
            nc.sync.dma_start(out=ssb[:], in_=sidx.ap())

            for _ in range(reps):
                tiles = {}

                def emit_gather(s):
                    t = data_pool.tile([P, 1, DIM], mybir.dt.float16)
                    tiles[s] = t
                    nc.gpsimd.dma_gather(
                        t[:],
                        bank_aps[slot_bank[s]],
                        gsb[:, s * IDX_COLS : (s + 1) * IDX_COLS],
                        P,
                        P,
                        DIM,
                        queue_num=s % N_QUEUES,
                    )

                def emit_scatter(s):
                    q = s % N_QUEUES
                    nc.gpsimd.dma_scatter_add(
                        outs[q].ap(),
                        tiles.pop(s)[:],
                        ssb[:, s * IDX_COLS : (s + 1) * IDX_COLS],
                        P,
                        P,
                        DIM,
                        queue_num=q,
                    )

                n_waves = (N_SLOTS + N_QUEUES - 1) // N_QUEUES
                for w in range(n_waves):
                    for s in range(w * N_QUEUES, min((w + 1) * N_QUEUES, N_SLOTS)):
                        emit_gather(s)
                    if w > 0:
                        for s in range((w - 1) * N_QUEUES, w * N_QUEUES):
                            emit_scatter(s)
                for s in range((n_waves - 1) * N_QUEUES, N_SLOTS):
                    emit_scatter(s)

    nc.compile()
    _NC_CACHE[key] = nc
    return nc


def _wrap16(a):
    """[N_SLOTS, 128] int16 -> [128, N_SLOTS*8]: idx i of slot s lands at
    (partition i%16, col s*8 + i//16), replicated to all 128 partitions."""
    blocks = a.reshape(N_SLOTS, IDX_COLS, 16).transpose(0, 2, 1)  # [S, 16, 8]
    flat = blocks.transpose(1, 0, 2).reshape(16, N_SLOTS * IDX_COLS)
    return np.ascontiguousarray(np.tile(flat, (8, 1)))


def _prep_core(ids_c):
    """Build gather/scatter index planes + host fixup list for one core."""
    bank = (ids_c >> 15).astype(np.int64)
    order = np.argsort(bank, kind="stable")
    gl = np.zeros((N_SLOTS, P), np.int16)      # pad: bank row 0 (valid)
    sl = np.full((N_SLOTS, P), S, np.int16)    # pad: dummy out row S
    fix = [np.where(ids_c >= OVER_START)[0]]   # reserved ids -> host fixup
    s0 = 0
    for b in range(N_BANKS):
        pos = order[bank[order] == b]
        spb = SLOTS_PER_BANK[b]
        if len(pos) > spb * P:  # static budget exceeded -> host fixup
            fix.append(pos[spb * P :])
            pos = pos[: spb * P]
        nch = (len(pos) + P - 1) // P
        for j in range(nch):
            pp = pos[j * P : (j + 1) * P]
            gl[s0 + j, : len(pp)] = (ids_c[pp] - (b << 15)).astype(np.int16)
            sl[s0 + j, : len(pp)] = pp.astype(np.int16)
        s0 += spb
    return _wrap16(gl), _wrap16(sl), np.unique(np.concatenate(fix))


def build_in_maps(input_ids, weight):
    """Per-core input maps (f16 table + index planes) + host fixup lists."""
    wtab = np.ascontiguousarray(weight.astype(np.float16))
    in_maps, fixes = [], []
    for c in range(N_CORES):
        g, sdx, fix = _prep_core(input_ids[c])
        in_maps.append({"wtab": wtab, "gidx": g, "sidx": sdx})
        fixes.append(fix)
    return in_maps, fixes


def kernel(input_ids, weight, extra):
    input_ids = np.ascontiguousarray(np.asarray(input_ids), dtype=np.int32)
    weight = np.ascontiguousarray(np.asarray(weight), dtype=np.float32)
    extra = np.ascontiguousarray(np.asarray(extra), dtype=np.float32)
    assert input_ids.shape == (B, S), input_ids.shape
    assert weight.shape == (VOCAB, DIM), weight.shape
    assert extra.shape == (N_OVER, DIM), extra.shape

    nc = _build_nc()
    in_maps, fixes = build_in_maps(input_ids, weight)

    res = run_bass_kernel_spmd(nc, in_maps, core_ids=list(range(N_CORES)))

    out = np.empty((B, S, DIM), np.float32)
    for c in range(N_CORES):
        r = res.results[c]
        acc = r["out0"][:S].astype(np.float32)
        for q in range(1, N_QUEUES):
            acc += r[f"out{q}"][:S].astype(np.float32)
        out[c] = acc
    # host fixup in full f32: reserved ids + any slot-budget overflow
    for c in range(N_CORES):
        fix = fixes[c]
        if len(fix) == 0:
            continue
        ids_f = input_ids[c][fix]
        rows = np.where(
            (ids_f >= OVER_START)[:, None],
            extra[np.clip(ids_f - OVER_START, 0, N_OVER - 1)],
            weight[ids_f],
        )
        out[c][fix] = rows
    return out
